# revision 19
# baseline (speedup 1.0000x reference)
"""FP4 (E2M1) per-tensor absmax fake-quantization on 8 TRN2 NeuronCores.

Strategy (data-parallel over 8 cores, shard = 2048 rows of x.reshape(16384, 4096)):
  Pass 1: stream shard tiles, DVE absmax reduce -> [1,1] local absmax
          -> AllReduce(max) -> global absmax
          -> s = max(absmax,1e-8)/6 (exact, via residual-corrected mul)
          -> c1 = (1/s)*2^-126, c2 = s*2^63 broadcast to 128 partitions
  Pass 2: stream shard tiles:
          z  = x * c1                      (ACT; z in [0,6]*2^-126, subnormal)
          zi = (bits(z) + 0x200000) & 0xFFC00000   (DVE; RNE-to-FP4-grid via
               fp32 subnormal behavior == E2M1 subnormal grid)
          q  = (z * 2^63) * c2             (DVE dual-scalar mult; == RN(L*s))
          DMA out.
All per-element steps are exact w.r.t. the reference up to the inherent
reciprocal-vs-division 1-2ulp difference (a few hundred single-level flips
out of 67M elements; L2 rel err ~1e-3).
"""
import sys
import os

for _p in ("/opt/trn_rl_repo", "/root/.axon_site/_ro/trn_rl_repo"):
    if os.path.isdir(_p) and _p not in sys.path:
        sys.path.insert(0, _p)

import numpy as np

NCORES = 8
ROWS, COLS = 16384, 4096          # x.reshape(16384, 4096)
SH_ROWS = ROWS // NCORES          # 2048 rows per core
P = 128                           # SBUF partitions
TILE_COLS = 2048                  # 1 MiB tiles
TILES = (SH_ROWS * COLS) // (P * TILE_COLS)   # 32 tiles per core
RES_TILES = 15                    # resident tiles (15 MB)
FULL_SHAPE = (4, 4096, 4096)

_cached = {}


def _build():
    import concourse.bass as bass
    from concourse import bacc
    import concourse.tile as tile
    import concourse.mybir as mybir
    from contextlib import ExitStack

    F32 = mybir.dt.float32
    I32 = mybir.dt.int32
    ts = bass.ts

    nc = bacc.Bacc("TRN2", target_bir_lowering=False, debug=False,
                   num_devices=NCORES)
    x = nc.dram_tensor("x", [SH_ROWS, COLS], F32, kind="ExternalInput").ap()
    out = nc.dram_tensor("out", [SH_ROWS, COLS], F32, kind="ExternalOutput").ap()
    cc_in = nc.dram_tensor("cc_in", [1, 1], F32)
    cc_out = nc.dram_tensor("cc_out", [1, 1], F32, addr_space="Shared")

    c6i = float(np.float32(1.0) / np.float32(6.0))
    AL = mybir.AluOpType

    def tile_src(i):
        # tile i -> [128, 2048] block of the [2048, 4096] shard
        r, c = divmod(i, COLS // TILE_COLS)
        return x[ts(r, P), ts(c, TILE_COLS)]

    def tile_dst(i):
        r, c = divmod(i, COLS // TILE_COLS)
        return out[ts(r, P), ts(c, TILE_COLS)]

    with tile.TileContext(nc) as tc:
        with ExitStack() as ctx:
            res = (ctx.enter_context(tc.tile_pool(name="res", bufs=RES_TILES))
                   if RES_TILES else None)
            st = ctx.enter_context(tc.tile_pool(name="st", bufs=6))
            stats = ctx.enter_context(tc.tile_pool(name="stats", bufs=1))

            # ---- Pass 1: local absmax (tiles 0..RES-1 stay resident) ----
            lmax = stats.tile([P, TILES], F32)
            resident = []
            for i in range(TILES):
                if i < RES_TILES:
                    t = res.tile([P, TILE_COLS], F32, tag="res")
                    resident.append(t)
                else:
                    t = st.tile([P, TILE_COLS], F32, tag="st")
                nc.sync.dma_start(t[:], tile_src(i))
                nc.vector.tensor_reduce(lmax[:, i:i + 1], t[:],
                                        mybir.AxisListType.X, AL.max,
                                        apply_absolute_value=True)
            # ---- Pass-2 reload DMAs: emitted BEFORE the collective's
            # critical sections (Tile gives pool allocations made after a
            # critical a sync-dependency on it, which would serialize the
            # prefetch behind the collective) ----
            p2_tiles = list(resident)
            for i in range(RES_TILES, TILES):
                t = st.tile([P, TILE_COLS], F32, tag="st")
                nc.sync.dma_start(t[:], tile_src(i))
                p2_tiles.append(t)

            lmax1 = stats.tile([P, 1], F32)
            nc.vector.tensor_reduce(lmax1[:], lmax[:], mybir.AxisListType.X,
                                    AL.max)
            g11 = stats.tile([1, 1], F32)
            nc.gpsimd.tensor_reduce(g11[:], lmax1[:], mybir.AxisListType.C,
                                    AL.max)

            # ---- AllReduce(max) over the 8 cores (no tile_critical:
            # criticals barrier SP/ACT via supported_engines, stalling the
            # prefetch stream; explicit dep edges instead) ----
            from concourse.tile_rust import add_dep_helper
            gmax = stats.tile([1, 1], F32)
            d1 = nc.gpsimd.dma_start(cc_in[:, :], g11[:])
            cc = nc.gpsimd.collective_compute(
                "AllReduce", AL.max,
                replica_groups=[list(range(NCORES))],
                ins=[cc_in.ap().opt()], outs=[cc_out.ap().opt()],
            )
            add_dep_helper(cc.ins, d1.ins, True, "cc after cc_in dma")
            d2 = nc.gpsimd.dma_start(gmax[:], cc_out[:, :])
            add_dep_helper(d2.ins, cc.ins, True, "gmax dma after cc done")

            # ---- scale constants (all [1,1], exact s = max(gmax,1e-8)/6) ----
            mt = stats.tile([1, 1], F32)
            nc.vector.tensor_scalar_max(mt[:], gmax[:], 1e-8)
            s0 = stats.tile([1, 1], F32)
            nc.vector.tensor_scalar_mul(s0[:], mt[:], c6i)
            t6 = stats.tile([1, 1], F32)
            nc.vector.tensor_scalar_mul(t6[:], s0[:], 6.0)
            w = stats.tile([1, 1], F32)
            nc.vector.scalar_tensor_tensor(w[:], s0[:], -4.0, t6[:],
                                           AL.mult, AL.add)
            d2 = stats.tile([1, 1], F32)
            nc.vector.scalar_tensor_tensor(d2[:], s0[:], 2.0, w[:],
                                           AL.mult, AL.subtract)
            e = stats.tile([1, 1], F32)
            nc.vector.scalar_tensor_tensor(e[:], t6[:], -1.0, mt[:],
                                           AL.mult, AL.add)
            resid = stats.tile([1, 1], F32)
            nc.vector.tensor_tensor(out=resid[:], in0=e[:], in1=d2[:],
                                    op=AL.subtract)
            sv = stats.tile([1, 1], F32)
            nc.vector.scalar_tensor_tensor(sv[:], resid[:], c6i, s0[:],
                                           AL.mult, AL.add)
            rr = stats.tile([1, 1], F32)
            nc.vector.reciprocal(rr[:], sv[:])
            # consts: [c1, c2b, c2f] = [r*2^-126, s*2^63, s*2^126]
            consts = stats.tile([1, 3], F32)
            nc.vector.tensor_scalar_mul(consts[:, 0:1], rr[:],
                                        float(2.0 ** -126))
            nc.vector.tensor_scalar_mul(consts[:, 1:2], sv[:],
                                        float(2.0 ** 63))
            nc.vector.tensor_scalar_mul(consts[:, 2:3], sv[:],
                                        float(2.0 ** 126))
            cb = stats.tile([P, 3], F32)
            nc.gpsimd.partition_broadcast(cb[:], consts[:])
            c1_ap = cb[:, 0:1]
            c2b_ap = cb[:, 1:2]
            c2f_ap = cb[:, 2:3]

            # ---- Pass 2: quantize (in-place); stores on gpsimd queue.
            # Resident tiles: batch all z's first so ACT's in-order queue
            # never stalls behind a final-mult waiting on DVE. ----
            def emit_z(t):
                nc.scalar.activation(t[:], t[:],
                                     mybir.ActivationFunctionType.Copy,
                                     scale=c1_ap)

            def emit_round(t):
                zi = t[:].bitcast(I32)
                nc.vector.tensor_scalar(zi, zi, 0x200000, None, AL.add)
                nc.vector.tensor_scalar(zi, zi, 0xFFC00000 - (1 << 32), None,
                                        AL.bitwise_and)

            def emit_mult_act(t):
                # q = Copy(c2f * h), c2f = s*2^126 (exact pow2 scaling)
                nc.scalar.activation(t[:], t[:],
                                     mybir.ActivationFunctionType.Copy,
                                     scale=c2f_ap)

            def emit_mult_dve(t):
                # q = (h * 2^63) * c2b
                nc.vector.tensor_scalar(t[:], t[:], float(2.0 ** 63),
                                        c2b_ap, AL.mult, AL.mult)

            LOOKAHEAD = 4
            for i in range(min(LOOKAHEAD, TILES)):
                emit_z(p2_tiles[i])
            for i in range(TILES):
                if i + LOOKAHEAD < TILES:
                    emit_z(p2_tiles[i + LOOKAHEAD])
                t = p2_tiles[i]
                emit_round(t)
                if i % 2 == 0:
                    emit_mult_act(t)
                else:
                    emit_mult_dve(t)
                nc.gpsimd.dma_start(tile_dst(i), t[:])

    nc.compile()
    return nc


def kernel(x):
    from concourse import bass_utils

    x = np.ascontiguousarray(np.asarray(x, dtype=np.float32))
    assert x.shape == FULL_SHAPE, x.shape

    if "nc" not in _cached:
        _cached["nc"] = _build()
    nc = _cached["nc"]

    flat = x.reshape(ROWS, COLS)
    in_maps = [{"x": flat[c * SH_ROWS:(c + 1) * SH_ROWS]} for c in range(NCORES)]
    res = bass_utils.run_bass_kernel_spmd(nc, in_maps,
                                          core_ids=list(range(NCORES)))
    out = np.concatenate([res.results[c]["out"] for c in range(NCORES)],
                         axis=0)
    return out.reshape(FULL_SHAPE)
